# revision 11
# baseline (speedup 1.0000x reference)
"""Trainium2 Bass kernel for the BDH recurrent block (B=8, T=256, d=256, n=1024).

One sample per NeuronCore (data-parallel over B=8), weights replicated.

The scan input v_prev is the *embedding* at each step (v_star is never fed
back), so the only recurrences are

  x_t  = (0.97 x_{t-1} + relu(emb_t Dx^T)) / b_t,  b_t = sum(U_t) + 0.97[t>0]
  rho_t = 0.97 rho_{t-1} + ln(emb_t) (x) x_t

Implementation notes:
 * x has closed form x_t = sum_s C[t,s] U_s with C[t,s] built from cumulative
   sums of log b (decay-masked).  Since the per-step carry weight 0.97/b_t is
   ~3e-3 (b in [323,505]), C is effectively banded; dropping the cross-block
   coupling makes C block-diagonal over two 128-step blocks (rel err ~3e-4),
   so X^T needs only 16 small 128-wide matmuls.
 * a*_t = rho_{t-1} x_t = ((X X^T) o Dup) @ ln(emb): decay-masked attention.
 * mean(a*) is exactly 0 (rows of ln(emb) are zero-mean), so the A-layernorm
   reduces to the per-row scale r_t = rsqrt(var+eps) (var=mean(a*^2) via the
   ACT Square+accumulate path); relu(r*c) = r*relu(c) lets r_t commute out
   to the v-matmul output where it's applied before the output layernorm.
 * A^T (for the Dy matmul) is computed by a second small matmul, no PE
   transposes.
 * bf16 storage for all big operands (halves HBM traffic; 8 cores share HBM).
 * Filler matmuls keep the PE busy through DMA/vector-chain gaps so the HAM
   clock gate stays at 2.4 GHz.
"""

import numpy as np
import ml_dtypes

import concourse.bass as bass
import concourse.tile as tile
from concourse import bacc, mybir
from concourse.bass_utils import run_bass_kernel_spmd
from concourse.hw_specs import get_activation_tables

B, T, D, N = 8, 256, 256, 1024
P = 128
LN_EPS = 1e-5
DECAY = 0.97
F32 = mybir.dt.float32
F32R = mybir.dt.float32r
BF16 = mybir.dt.bfloat16
AF = mybir.ActivationFunctionType
ALU = mybir.AluOpType
NPBF16 = ml_dtypes.bfloat16

FILL_PRE = 16
FILL_CHAIN = 4
FILL_XT = 3
FILL_GD = 3
FILL_AT = 2

# dup-pack column offsets (f32 consts tensor [P, 774])
O_DUP0 = 0
O_DUP1 = 256
O_TRIU = 384
O_IDENT = 512
O_MTRI = 640
O_IOTP = 768   # iotaP cols (2)
O_IOTQ = 770   # iotaQ cols (2)
O_C097 = 772   # c097 cols (2)
DUPW = 774


def _build_nc():
    nc = bacc.Bacc(enable_partition_id=False)

    d_embdx0 = nc.dram_tensor("embdx0", [P, 1280], BF16, kind="ExternalInput")
    d_embdx1 = nc.dram_tensor("embdx1", [P, 1280], BF16, kind="ExternalInput")
    d_et = nc.dram_tensor("et", [P, 2048], BF16, kind="ExternalInput")
    d_dyt = nc.dram_tensor("dyt", [P, 2048], BF16, kind="ExternalInput")
    d_emb = nc.dram_tensor("emb", [P, 512], BF16, kind="ExternalInput")
    d_dup = nc.dram_tensor("dup", [P, DUPW], F32, kind="ExternalInput")
    d_out = nc.dram_tensor("out", [T, D], F32, kind="ExternalOutput")

    act_sets = list(get_activation_tables(nc.m.arch))
    combined_set_id = act_sets.index("natural_log_exp_and_others")

    with tile.TileContext(nc) as tc:
        nc.scalar.add_instruction(mybir.InstLoadActFuncSet(
            name=nc.get_next_instruction_name(),
            act_func_set_id=combined_set_id, ins=[], outs=[]))
        with (
            tc.tile_pool(name="consts", bufs=1) as cp,
            tc.tile_pool(name="work", bufs=1) as wp,
            tc.tile_pool(name="ps512", bufs=2, space="PSUM") as ps512,
            tc.tile_pool(name="ps256", bufs=4, space="PSUM") as ps256,
            tc.tile_pool(name="pss", bufs=1, space="PSUM") as pss,
            tc.tile_pool(name="psF", bufs=1, space="PSUM") as psF,
        ):
            # ---- input DMAs (3 queues; first chunks gate the U matmuls) --
            embdx0 = cp.tile([P, 1280], BF16, tag="embdx0", name="embdx0")
            embdx1 = cp.tile([P, 1280], BF16, tag="embdx1", name="embdx1")
            et_big = cp.tile([P, 2048], BF16, tag="et", name="et")
            dyt = cp.tile([P, 2048], BF16, tag="dyt", name="dyt")
            embp = cp.tile([P, 512], BF16, tag="embp", name="embp")
            dup = cp.tile([P, DUPW], F32, tag="dup", name="dup")
            nc.sync.dma_start(embdx0[:], d_embdx0[:, :])
            nc.scalar.dma_start(embdx1[:], d_embdx1[:, :])
            nc.sync.dma_start(et_big[:], d_et[:, :])
            nc.scalar.dma_start(dyt[:], d_dyt[:, :])
            nc.gpsimd.dma_start(embp[:], d_emb[:, :])
            nc.gpsimd.dma_start(dup[:], d_dup[:, :])

            embT = [embdx0[:, 0:T], embdx1[:, 0:T]]
            DxT = [embdx0[:, T:1280], embdx1[:, T:1280]]
            DyT = [dyt[:, 0:N], dyt[:, N:2 * N]]
            ET = [et_big[:, m * D:(m + 1) * D] for m in range(8)]
            emb_s = [embp[:, 0:D], embp[:, D:2 * D]]
            Dup0 = dup[:, O_DUP0:O_DUP0 + T]
            Dup1r = dup[:, O_DUP1:O_DUP1 + P]
            triu_s = dup[:, O_TRIU:O_TRIU + P]
            ident_s = dup[:, O_IDENT:O_IDENT + P]
            Mtri = dup[:, O_MTRI:O_MTRI + P]
            iotaP_c = [dup[:, O_IOTP + k:O_IOTP + k + 1] for k in range(2)]
            iotaQ_c = [dup[:, O_IOTQ + k:O_IOTQ + k + 1] for k in range(2)]
            c097_c = [dup[:, O_C097 + k:O_C097 + k + 1] for k in range(2)]

            # ---- small consts via memset (DVE: earliest idle engine) -----
            scr_l = cp.tile([P, P], BF16, tag="scr_l", name="scr_l")
            nc.vector.memset(scr_l[:], 0.25)
            scr_r = cp.tile([P, T], BF16, tag="scr_r", name="scr_r")
            nc.vector.memset(scr_r[:], 0.25)
            eps_col = cp.tile([P, 1], F32, tag="eps_col", name="eps_col")
            nc.vector.memset(eps_col[:], LN_EPS)
            zero_col = cp.tile([P, 1], F32, tag="zero_col", name="zero_col")
            nc.vector.memset(zero_col[:], 0.0)
            ones_rowf = cp.tile([1, P], F32, tag="ones_rowf", name="ones_rowf")
            nc.vector.memset(ones_rowf[:], 1.0)
            ones_blk = cp.tile([P, P], F32, tag="ones_blk", name="ones_blk")
            nc.vector.memset(ones_blk[:], 1.0)
            GD1 = wp.tile([P, T], BF16, tag="GD1", name="GD1")
            nc.gpsimd.memset(GD1[:], 0.0)

            fill_ps = psF.tile([P, T], F32, tag="fill", name="fill")

            def fillers(k):
                for _ in range(k):
                    nc.tensor.matmul(fill_ps[:], scr_l[:], scr_r[:],
                                     start=True, stop=True)

            fillers(FILL_PRE)

            # ---- U = relu(emb Dx^T) in [t, n] layout; b via accum --------
            U = [wp.tile([P, N], BF16, tag=f"U{mt}", name=f"U{mt}")
                 for mt in range(2)]
            apart = [[wp.tile([P, 1], F32, tag=f"ap{mt}{ch}",
                              name=f"ap{mt}{ch}") for ch in range(2)]
                     for mt in range(2)]
            for mt in range(2):
                for ch in range(2):
                    pu = ps512.tile([P, 512], F32, tag="pu", name=f"pu{mt}{ch}")
                    for k in range(2):
                        nc.tensor.matmul(
                            pu[:], embT[k][:, mt * P:(mt + 1) * P],
                            DxT[k][:, ch * 512:(ch + 1) * 512],
                            start=(k == 0), stop=(k == 1))
                    if ch == 0:
                        nc.scalar.activation(
                            out=U[mt][:, ch * 512:(ch + 1) * 512], in_=pu[:],
                            func=AF.Relu, bias=zero_col[:],
                            accum_out=apart[mt][ch][:])
                    else:
                        nc.vector.tensor_scalar(
                            U[mt][:, ch * 512:(ch + 1) * 512], pu[:], 0.0,
                            0.0, op0=ALU.max, op1=ALU.add,
                            accum_out=apart[mt][ch][:])

            # ---- b, log b, cumsum (PE), p/q ------------------------------
            logb = []
            for mt in range(2):
                bvec = wp.tile([P, 1], F32, tag=f"b{mt}", name=f"b{mt}")
                nc.vector.scalar_tensor_tensor(
                    out=bvec[:], in0=apart[mt][0][:], scalar=c097_c[mt],
                    in1=apart[mt][1][:], op0=ALU.add, op1=ALU.add)
                lb = wp.tile([P, 1], F32, tag=f"lb{mt}", name=f"lb{mt}")
                nc.scalar.activation(out=lb[:], in_=bvec[:], func=AF.Ln,
                                     bias=zero_col[:])
                logb.append(lb)
            pl_tiles = []
            for mt in range(2):
                pl = pss.tile([P, 1], F32, tag="pss", name=f"pl{mt}")
                if mt == 0:
                    nc.tensor.matmul(pl[:], triu_s, logb[0][:],
                                     start=True, stop=True)
                else:
                    nc.tensor.matmul(pl[:], ones_blk[:], logb[0][:],
                                     start=True, stop=False)
                    nc.tensor.matmul(pl[:], triu_s, logb[1][:],
                                     start=False, stop=True)
                pl_tiles.append(pl)
            fillers(FILL_CHAIN)
            q_col = []
            p_col = []
            for mt in range(2):
                qv = wp.tile([P, 1], F32, tag=f"q{mt}", name=f"q{mt}")
                nc.vector.tensor_tensor(qv[:], pl_tiles[mt][:], iotaQ_c[mt],
                                        op=ALU.add)
                q_col.append(qv)
                pv_ = wp.tile([P, 1], F32, tag=f"p{mt}", name=f"p{mt}")
                nc.vector.scalar_tensor_tensor(
                    out=pv_[:], in0=iotaP_c[mt], scalar=pl_tiles[mt][:],
                    in1=logb[mt][:], op0=ALU.subtract, op1=ALU.subtract)
                p_col.append(pv_)

            # ---- p as row (PE transpose), broadcast, CT = exp ------------
            p_row = wp.tile([1, T], F32, tag="p_row", name="p_row")
            for mt in range(2):
                pt = pss.tile([1, P], F32, tag="pss", name=f"pt{mt}")
                nc.tensor.transpose(pt[:], p_col[mt][:], ident_s)
                nc.vector.tensor_copy(p_row[:, mt * P:(mt + 1) * P], pt[:])
            pb = ps256.tile([P, T], F32, tag="ps", name="pb")
            nc.tensor.matmul(pb[:], ones_rowf[:], p_row[:],
                             start=True, stop=True)
            fillers(FILL_XT)
            ct = []
            for k in range(2):
                tmp = wp.tile([P, P], F32, tag=f"ctmp{k}", name=f"ctmp{k}")
                nc.vector.tensor_tensor(tmp[:], pb[:, k * P:(k + 1) * P],
                                        Mtri, op=ALU.add)
                c = wp.tile([P, P], BF16, tag=f"ct{k}", name=f"ct{k}")
                nc.scalar.activation(out=c[:], in_=tmp[:], func=AF.Exp,
                                     bias=q_col[k][:])
                ct.append(c)

            # ---- X^T: block-diagonal C -> 16 128-wide matmuls ------------
            XT = [wp.tile([P, T], F32R, tag=f"XT{m}", name=f"XT{m}")
                  for m in range(8)]
            for m in range(8):
                px = ps256.tile([P, T], F32, tag="ps", name=f"px{m}")
                for k in range(2):
                    nc.tensor.matmul(px[:, k * P:(k + 1) * P],
                                     U[k][:, m * P:(m + 1) * P], ct[k][:],
                                     start=True, stop=True)
                if m % 2 == 0:
                    nc.scalar.copy(XT[m][:], px[:])
                else:
                    nc.vector.tensor_copy(XT[m][:], px[:])

            # ---- G = X X^T ; GD = G o Dup --------------------------------
            pg0 = ps256.tile([P, T], F32, tag="ps", name="pg0")
            pg1 = ps256.tile([P, T], F32, tag="ps", name="pg1")
            for m in range(8):
                nc.tensor.matmul(pg0[:], XT[m][:, 0:P], XT[m][:],
                                 start=(m == 0), stop=(m == 7))
                nc.tensor.matmul(pg1[:], XT[m][:, P:T], XT[m][:],
                                 start=(m == 0), stop=(m == 7))
            fillers(FILL_GD)
            GD0 = wp.tile([P, T], BF16, tag="GD0", name="GD0")
            nc.vector.tensor_tensor(GD0[:], pg0[:], Dup0, op=ALU.mult)
            nc.vector.tensor_tensor(GD1[:, P:T], pg1[:, P:T], Dup1r,
                                    op=ALU.mult)

            # ---- W = ln(emb rows)  (early, overlaps U/chain) -------------
            W = []
            for mt in range(2):
                st6 = wp.tile([P, 6], F32, tag=f"wst{mt}", name=f"wst{mt}")
                nc.vector.bn_stats(st6[:], emb_s[mt])
                mv = wp.tile([P, 2], F32, tag=f"wmv{mt}", name=f"wmv{mt}")
                nc.vector.bn_aggr(mv[:], st6[:])
                lv = wp.tile([P, 1], F32, tag=f"wlv{mt}", name=f"wlv{mt}")
                nc.scalar.activation(out=lv[:], in_=mv[:, 1:2], func=AF.Ln,
                                     bias=eps_col[:])
                rs = wp.tile([P, 1], F32, tag=f"wrs{mt}", name=f"wrs{mt}")
                nc.scalar.activation(out=rs[:], in_=lv[:], func=AF.Exp,
                                     bias=zero_col[:], scale=-0.5)
                w = wp.tile([P, D], BF16, tag=f"W{mt}", name=f"W{mt}")
                nc.vector.tensor_scalar(w[:], emb_s[mt], mv[:, 0:1], rs[:],
                                        op0=ALU.subtract, op1=ALU.mult)
                W.append(w)

            # ---- A^T directly (no transposes); pa for var only -----------
            ATp = [ps256.tile([P, T], F32, tag="ps", name=f"ATp{dt}")
                   for dt in range(2)]
            for dt in range(2):
                nc.tensor.matmul(ATp[dt][:], W[0][:, dt * P:(dt + 1) * P],
                                 GD0[:], start=True, stop=False)
                nc.tensor.matmul(ATp[dt][:], W[1][:, dt * P:(dt + 1) * P],
                                 GD1[:], start=False, stop=True)
            pa = [ps256.tile([P, D], F32, tag="ps", name=f"pa{mt}")
                  for mt in range(2)]
            nc.tensor.matmul(pa[0][:], GD0[:, 0:P], W[0][:],
                             start=True, stop=True)
            nc.tensor.matmul(pa[1][:], GD0[:, P:T], W[0][:],
                             start=True, stop=False)
            nc.tensor.matmul(pa[1][:], GD1[:, P:T], W[1][:],
                             start=False, stop=True)
            fillers(FILL_AT)

            AT = [wp.tile([P, T], BF16, tag=f"AT{dt}", name=f"AT{dt}")
                  for dt in range(2)]
            nc.scalar.copy(AT[0][:], ATp[0][:])
            nc.vector.tensor_copy(AT[1][:], ATp[1][:])

            # r_t = rsqrt(var(a*_t)+eps); mean(a*)==0, var=sum(a^2)/D via ACT
            r_col = []
            for mt in range(2):
                sq = wp.tile([P, D], F32, tag=f"asq{mt}", name=f"asq{mt}")
                ss = wp.tile([P, 1], F32, tag=f"ass{mt}", name=f"ass{mt}")
                nc.scalar.activation(out=sq[:], in_=pa[mt][:], func=AF.Square,
                                     bias=zero_col[:], accum_out=ss[:])
                lv = wp.tile([P, 1], F32, tag=f"alv{mt}", name=f"alv{mt}")
                nc.scalar.activation(out=lv[:], in_=ss[:], func=AF.Ln,
                                     bias=eps_col[:], scale=1.0 / D)
                rr = wp.tile([P, 1], F32, tag=f"ar{mt}", name=f"ar{mt}")
                nc.scalar.activation(out=rr[:], in_=lv[:], func=AF.Exp,
                                     bias=zero_col[:], scale=-0.5)
                r_col.append(rr)
            r2_col = []
            for mt in range(2):
                r2 = wp.tile([P, 1], F32, tag=f"r2{mt}", name=f"r2{mt}")
                nc.vector.tensor_tensor(r2[:], r_col[mt][:], r_col[mt][:],
                                        op=ALU.mult)
                r2_col.append(r2)

            # ---- y^T = relu(Dy A^T) o X^T --------------------------------
            yT = [wp.tile([P, T], BF16, tag=f"yT{m}", name=f"yT{m}")
                  for m in range(8)]
            for m in range(8):
                py = ps256.tile([P, T], F32, tag="ps", name=f"py{m}")
                for k in range(2):
                    nc.tensor.matmul(py[:], DyT[k][:, m * P:(m + 1) * P],
                                     AT[k][:], start=(k == 0), stop=(k == 1))
                if m % 2 == 0:
                    yb = wp.tile([P, T], F32, tag=f"yb{m}", name=f"yb{m}")
                    nc.scalar.activation(out=yb[:], in_=py[:], func=AF.Relu,
                                         bias=zero_col[:])
                    nc.gpsimd.tensor_tensor(yT[m][:], yb[:],
                                            XT[m][:].bitcast(F32),
                                            op=ALU.mult)
                else:
                    nc.vector.scalar_tensor_tensor(
                        out=yT[m][:], in0=py[:], scalar=0.0,
                        in1=XT[m][:].bitcast(F32),
                        op0=ALU.max, op1=ALU.mult)

            # ---- v = y E^T; apply r_t inside the output layernorm --------
            for mt in range(2):
                pv = ps256.tile([P, D], F32, tag="ps", name=f"pv{mt}")
                for m in range(8):
                    nc.tensor.matmul(pv[:], yT[m][:, mt * P:(mt + 1) * P],
                                     ET[m], start=(m == 0), stop=(m == 7))
                st6 = wp.tile([P, 6], F32, tag=f"ost{mt}", name=f"ost{mt}")
                nc.vector.bn_stats(st6[:], pv[:])
                mv = wp.tile([P, 2], F32, tag=f"omv{mt}", name=f"omv{mt}")
                nc.vector.bn_aggr(mv[:], st6[:])
                # out = (pv - mean) * r * rsqrt(r^2 var + eps)
                t1 = wp.tile([P, 1], F32, tag=f"ot1{mt}", name=f"ot1{mt}")
                nc.vector.tensor_tensor(t1[:], mv[:, 1:2], r2_col[mt][:],
                                        op=ALU.mult)
                lv = wp.tile([P, 1], F32, tag=f"olv{mt}", name=f"olv{mt}")
                nc.scalar.activation(out=lv[:], in_=t1[:], func=AF.Ln,
                                     bias=eps_col[:])
                rq = wp.tile([P, 1], F32, tag=f"orq{mt}", name=f"orq{mt}")
                nc.scalar.activation(out=rq[:], in_=lv[:], func=AF.Exp,
                                     bias=zero_col[:], scale=-0.5)
                s = wp.tile([P, 1], F32, tag=f"os{mt}", name=f"os{mt}")
                nc.vector.tensor_tensor(s[:], rq[:], r_col[mt][:],
                                        op=ALU.mult)
                ov = wp.tile([P, D], F32, tag=f"ov{mt}", name=f"ov{mt}")
                nc.vector.tensor_scalar(ov[:], pv[:], mv[:, 0:1], s[:],
                                        op0=ALU.subtract, op1=ALU.mult)
                eng_out = nc.sync if mt == 0 else nc.scalar
                eng_out.dma_start(d_out[mt * P:(mt + 1) * P, :], ov[:])

    nc.finalize()
    return nc


_NC_CACHE = {}


def _get_nc(use_f32r=True):
    if "nc" not in _NC_CACHE:
        _NC_CACHE["nc"] = _build_nc()
    return _NC_CACHE["nc"]


def _host_consts():
    ii = np.arange(T, dtype=np.float64)
    ln097 = np.log(np.float64(DECAY))
    DupT = np.where(
        ii[:, None] < ii[None, :],
        np.float64(DECAY) ** (ii[None, :] - 1 - ii[:, None]),
        0.0,
    ).astype(np.float32)
    jj = np.arange(P)
    triu = np.triu(np.ones((P, P), np.float32), k=1)
    ident = np.eye(P, dtype=np.float32)
    mtri = np.where(jj[:, None] <= jj[None, :], 0.0, -1e30).astype(np.float32)
    iotp = np.stack([(jj + mt * P) * ln097 for mt in range(2)],
                    axis=1).astype(np.float32)
    iotq = (-iotp).astype(np.float32)
    c097 = np.full((P, 2), DECAY, np.float32)
    c097[0, 0] = 0.0
    pack = np.concatenate(
        [DupT[0:P, 0:T], DupT[P:T, P:T], triu, ident, mtri, iotp, iotq,
         c097], axis=1)
    assert pack.shape == (P, DUPW), pack.shape
    return np.ascontiguousarray(pack)


def make_in_maps(embeddings, E, Dx, Dy):
    emb = np.asarray(embeddings, dtype=np.float32)
    E = np.asarray(E, dtype=np.float32)
    Dx = np.asarray(Dx, dtype=np.float32)
    Dy = np.asarray(Dy, dtype=np.float32)
    dup_pack = _host_consts()
    DxT = Dx.T  # [d, n]
    DyTp = np.ascontiguousarray(
        Dy.T.reshape(2, P, N).transpose(1, 0, 2).reshape(P, 2 * N)
    ).astype(NPBF16)
    ETp = np.ascontiguousarray(
        E.T.reshape(8, P, D).transpose(1, 0, 2).reshape(P, 8 * D)
    ).astype(NPBF16)
    shared = {"dyt": DyTp, "et": ETp, "dup": dup_pack}
    in_maps = []
    for b in range(B):
        m = dict(shared)
        embT = emb[b].T  # [d, t]
        for k in range(2):
            m[f"embdx{k}"] = np.ascontiguousarray(np.concatenate(
                [embT[k * P:(k + 1) * P, :], DxT[k * P:(k + 1) * P, :]],
                axis=1)).astype(NPBF16)
        m["emb"] = np.ascontiguousarray(
            emb[b].reshape(2, P, D).transpose(1, 0, 2).reshape(P, 2 * D)
        ).astype(NPBF16)
        in_maps.append(m)
    return in_maps


def kernel(embeddings, E, Dx, Dy, _use_f32r=True):
    in_maps = make_in_maps(embeddings, E, Dx, Dy)
    nc = _get_nc()
    res = run_bass_kernel_spmd(nc, in_maps, core_ids=list(range(B)))
    return np.stack([r["out"] for r in res.results], axis=0)


# revision 16
# speedup vs baseline: 1.1003x; 1.1003x over previous
"""Trainium2 Bass kernel for the BDH recurrent block (B=8, T=256, d=256, n=1024).

One sample per NeuronCore (data-parallel over B=8), weights replicated.

The scan input v_prev is the *embedding* at each step (v_star is never fed
back), so the only recurrences are

  x_t  = (0.97 x_{t-1} + relu(emb_t Dx^T)) / b_t,  b_t = sum(U_t) + 0.97[t>0]
  rho_t = 0.97 rho_{t-1} + ln(emb_t) (x) x_t

Implementation notes:
 * x has closed form x_t = sum_s C[t,s] U_s with C[t,s] built from cumulative
   sums of log b (decay-masked).  Since the per-step carry weight 0.97/b_t is
   ~3e-3 (b in [323,505]), C is effectively banded; dropping the cross-block
   coupling makes C block-diagonal over two 128-step blocks (rel err ~3e-4),
   so X^T needs only 16 small 128-wide matmuls.
 * a*_t = rho_{t-1} x_t = ((X X^T) o Dup) @ ln(emb): decay-masked attention.
 * mean(a*) is exactly 0 (rows of ln(emb) are zero-mean), so the A-layernorm
   reduces to the per-row scale r_t = rsqrt(var+eps) (var=mean(a*^2) via the
   ACT Square+accumulate path); relu(r*c) = r*relu(c) lets r_t commute out
   to the v-matmul output where it's applied before the output layernorm.
 * A^T (for the Dy matmul) is computed by a second small matmul, no PE
   transposes.
 * bf16 storage for all big operands (halves HBM traffic; 8 cores share HBM).
 * Filler matmuls keep the PE busy through DMA/vector-chain gaps so the HAM
   clock gate stays at 2.4 GHz.
"""

import numpy as np
import ml_dtypes

import concourse.bass as bass
import concourse.tile as tile
from concourse import bacc, mybir
from concourse.bass_utils import run_bass_kernel_spmd
from concourse.hw_specs import get_activation_tables

B, T, D, N = 8, 256, 256, 1024
P = 128
LN_EPS = 1e-5
DECAY = 0.97
F32 = mybir.dt.float32
F32R = mybir.dt.float32r
BF16 = mybir.dt.bfloat16
AF = mybir.ActivationFunctionType
ALU = mybir.AluOpType
NPBF16 = ml_dtypes.bfloat16

FILL_PRE = 16
FILL_CHAIN = 4
FILL_XT = 3
FILL_GD = 3
FILL_AT = 2

# dup-pack column offsets (f32 consts tensor [P, 774])
O_DUP0 = 0
O_DUP1 = 256
O_TRIU = 384
O_IDENT = 512
O_MTRI = 640
O_IOTP = 768   # iotaP cols (2)
O_IOTQ = 770   # iotaQ cols (2)
O_C097 = 772   # c097 cols (2)
DUPW = 774


def _build_nc():
    nc = bacc.Bacc(enable_partition_id=False)

    d_embdx0 = nc.dram_tensor("embdx0", [P, 1280], BF16, kind="ExternalInput")
    d_embdx1 = nc.dram_tensor("embdx1", [P, 1280], BF16, kind="ExternalInput")
    d_et = nc.dram_tensor("et", [P, 2048], BF16, kind="ExternalInput")
    d_dyt = nc.dram_tensor("dyt", [P, 2048], BF16, kind="ExternalInput")
    d_emb = nc.dram_tensor("emb", [P, 512], BF16, kind="ExternalInput")
    d_dup = nc.dram_tensor("dup", [P, DUPW], F32, kind="ExternalInput")
    d_out = nc.dram_tensor("out", [T, D], F32, kind="ExternalOutput")

    act_sets = list(get_activation_tables(nc.m.arch))
    combined_set_id = act_sets.index("natural_log_exp_and_others")

    with tile.TileContext(nc) as tc:
        nc.scalar.add_instruction(mybir.InstLoadActFuncSet(
            name=nc.get_next_instruction_name(),
            act_func_set_id=combined_set_id, ins=[], outs=[]))
        with (
            tc.tile_pool(name="consts", bufs=1) as cp,
            tc.tile_pool(name="work", bufs=1) as wp,
            tc.tile_pool(name="ps512", bufs=2, space="PSUM") as ps512,
            tc.tile_pool(name="ps256", bufs=4, space="PSUM") as ps256,
            tc.tile_pool(name="pss", bufs=1, space="PSUM") as pss,
            tc.tile_pool(name="psF", bufs=1, space="PSUM") as psF,
        ):
            # ---- input DMAs (3 queues; first chunks gate the U matmuls) --
            embdx0 = cp.tile([P, 1280], BF16, tag="embdx0", name="embdx0")
            embdx1 = cp.tile([P, 1280], BF16, tag="embdx1", name="embdx1")
            et_big = cp.tile([P, 2048], BF16, tag="et", name="et")
            dyt = cp.tile([P, 2048], BF16, tag="dyt", name="dyt")
            embp = cp.tile([P, 512], BF16, tag="embp", name="embp")
            dup = cp.tile([P, DUPW], F32, tag="dup", name="dup")
            nc.sync.dma_start(embdx0[:], d_embdx0[:, :])
            nc.scalar.dma_start(embdx1[:], d_embdx1[:, :])
            nc.gpsimd.dma_start(dup[:], d_dup[:, :])
            nc.gpsimd.dma_start(embp[:], d_emb[:, :])
            nc.sync.dma_start(et_big[:], d_et[:, :])
            nc.scalar.dma_start(dyt[:], d_dyt[:, :])

            embT = [embdx0[:, 0:T], embdx1[:, 0:T]]
            DxT = [embdx0[:, T:1280], embdx1[:, T:1280]]
            DyT = [dyt[:, 0:N], dyt[:, N:2 * N]]
            ET = [et_big[:, m * D:(m + 1) * D] for m in range(8)]
            emb_s = [embp[:, 0:D], embp[:, D:2 * D]]
            Dup0 = dup[:, O_DUP0:O_DUP0 + T]
            Dup1r = dup[:, O_DUP1:O_DUP1 + P]
            triu_s = dup[:, O_TRIU:O_TRIU + P]
            ident_s = dup[:, O_IDENT:O_IDENT + P]
            Mtri = dup[:, O_MTRI:O_MTRI + P]
            iotaP2 = dup[:, O_IOTP:O_IOTP + 2]
            iotaQ2 = dup[:, O_IOTQ:O_IOTQ + 2]
            c097_c = [dup[:, O_C097 + k:O_C097 + k + 1] for k in range(2)]

            # ---- small consts via memset (DVE: earliest idle engine) -----
            scr_l = cp.tile([P, P], BF16, tag="scr_l", name="scr_l")
            nc.vector.memset(scr_l[:], 0.25)
            scr_r = cp.tile([P, T], BF16, tag="scr_r", name="scr_r")
            nc.vector.memset(scr_r[:], 0.25)
            eps_col = cp.tile([P, 1], F32, tag="eps_col", name="eps_col")
            nc.vector.memset(eps_col[:], LN_EPS)
            zero_col = cp.tile([P, 1], F32, tag="zero_col", name="zero_col")
            nc.vector.memset(zero_col[:], 0.0)
            ones_rowf = cp.tile([1, P], F32, tag="ones_rowf", name="ones_rowf")
            nc.vector.memset(ones_rowf[:], 1.0)
            ones_blk = cp.tile([P, P], F32, tag="ones_blk", name="ones_blk")
            nc.vector.memset(ones_blk[:], 1.0)
            ones33 = cp.tile([33, P], F32, tag="ones33", name="ones33")
            nc.vector.memset(ones33[:], 1.0)
            GD1 = wp.tile([P, T], BF16, tag="GD1", name="GD1")
            nc.gpsimd.memset(GD1[:], 0.0)

            fill_ps = psF.tile([P, T], F32, tag="fill", name="fill")

            def fillers(k):
                for _ in range(k):
                    nc.tensor.matmul(fill_ps[:], scr_l[:], scr_r[:],
                                     start=True, stop=True)

            fillers(FILL_PRE)

            # ---- U = relu(emb Dx^T) in [t, n] layout; b via accum --------
            U = [wp.tile([P, N], BF16, tag=f"U{mt}", name=f"U{mt}")
                 for mt in range(2)]
            apart = [[wp.tile([P, 1], F32, tag=f"ap{mt}{ch}",
                              name=f"ap{mt}{ch}") for ch in range(2)]
                     for mt in range(2)]
            for mt in range(2):
                pus = [ps512.tile([P, 512], F32, tag="pu", name=f"pu{mt}{ch}")
                       for ch in range(2)]
                for k in range(2):
                    for ch in range(2):
                        nc.tensor.matmul(
                            pus[ch][:], embT[k][:, mt * P:(mt + 1) * P],
                            DxT[k][:, ch * 512:(ch + 1) * 512],
                            start=(k == 0), stop=(k == 1))
                for ch in range(2):
                    if ch == 0:
                        nc.scalar.activation(
                            out=U[mt][:, ch * 512:(ch + 1) * 512],
                            in_=pus[ch][:], func=AF.Relu, bias=zero_col[:],
                            accum_out=apart[mt][ch][:])
                    else:
                        nc.vector.tensor_scalar(
                            U[mt][:, ch * 512:(ch + 1) * 512], pus[ch][:],
                            0.0, 0.0, op0=ALU.max, op1=ALU.add,
                            accum_out=apart[mt][ch][:])

            # ---- b, log b, cumsum (PE), p/q ------------------------------
            logb = []
            for mt in range(2):
                bvec = wp.tile([P, 1], F32, tag=f"b{mt}", name=f"b{mt}")
                nc.vector.scalar_tensor_tensor(
                    out=bvec[:], in0=apart[mt][0][:], scalar=c097_c[mt],
                    in1=apart[mt][1][:], op0=ALU.add, op1=ALU.add)
                lb = wp.tile([P, 1], F32, tag=f"lb{mt}", name=f"lb{mt}")
                nc.scalar.activation(out=lb[:], in_=bvec[:], func=AF.Ln,
                                     bias=zero_col[:])
                logb.append(lb)
            pl_tiles = []
            for mt in range(2):
                pl = pss.tile([P, 1], F32, tag="pss", name=f"pl{mt}")
                if mt == 0:
                    nc.tensor.matmul(pl[:], triu_s, logb[0][:],
                                     start=True, stop=True)
                else:
                    nc.tensor.matmul(pl[:], ones_blk[:], logb[0][:],
                                     start=True, stop=False)
                    nc.tensor.matmul(pl[:], triu_s, logb[1][:],
                                     start=False, stop=True)
                pl_tiles.append(pl)
            fillers(FILL_CHAIN)
            q_col = []
            p_col = []
            for mt in range(2):
                qv = wp.tile([P, 1], F32, tag=f"q{mt}", name=f"q{mt}")
                nc.vector.tensor_tensor(qv[:], pl_tiles[mt][:],
                                        iotaQ2[:, mt:mt + 1], op=ALU.add)
                q_col.append(qv)
                pv_ = wp.tile([P, 1], F32, tag=f"p{mt}", name=f"p{mt}")
                nc.vector.scalar_tensor_tensor(
                    out=pv_[:], in0=iotaP2[:, mt:mt + 1],
                    scalar=pl_tiles[mt][:], in1=logb[mt][:],
                    op0=ALU.subtract, op1=ALU.subtract)
                p_col.append(pv_)

            # ---- p as row (PE transpose), broadcast, CT = exp ------------
            p_row = wp.tile([1, T], F32, tag="p_row", name="p_row")
            for mt in range(2):
                pt = pss.tile([1, P], F32, tag="pss", name=f"pt{mt}")
                nc.tensor.transpose(pt[:], p_col[mt][:], ident_s)
                nc.vector.tensor_copy(p_row[:, mt * P:(mt + 1) * P], pt[:])
            pb = ps256.tile([P, T], F32, tag="ps", name="pb")
            nc.tensor.matmul(pb[:], ones_rowf[:], p_row[:],
                             start=True, stop=True)
            fillers(FILL_XT)
            ct = []
            for k in range(2):
                tmp = wp.tile([P, P], F32, tag=f"ctmp{k}", name=f"ctmp{k}")
                nc.vector.tensor_tensor(tmp[:], pb[:, k * P:(k + 1) * P],
                                        Mtri, op=ALU.add)
                c = wp.tile([P, P], BF16, tag=f"ct{k}", name=f"ct{k}")
                nc.scalar.activation(out=c[:], in_=tmp[:], func=AF.Exp,
                                     bias=q_col[k][:])
                ct.append(c)

            # ---- X^T: block-diagonal C -> 16 128-wide matmuls ------------
            XT = [wp.tile([P, T], BF16, tag=f"XT{m}", name=f"XT{m}")
                  for m in range(8)]
            for m in range(8):
                px = ps256.tile([P, T], F32, tag="ps", name=f"px{m}")
                for k in range(2):
                    nc.tensor.matmul(px[:, k * P:(k + 1) * P],
                                     U[k][:, m * P:(m + 1) * P], ct[k][:],
                                     start=True, stop=True)
                if m % 2 == 0:
                    nc.scalar.copy(XT[m][:], px[:])
                else:
                    nc.vector.tensor_copy(XT[m][:], px[:])

            # ---- G = X X^T ; GD = G o Dup --------------------------------
            pg0 = ps256.tile([P, T], F32, tag="ps", name="pg0")
            pg1 = ps256.tile([P, T], F32, tag="ps", name="pg1")
            for m in range(8):
                nc.tensor.matmul(pg0[:], XT[m][:, 0:P], XT[m][:],
                                 start=(m == 0), stop=(m == 7))
                nc.tensor.matmul(pg1[:], XT[m][:, P:T], XT[m][:],
                                 start=(m == 0), stop=(m == 7))
            fillers(FILL_GD)
            GD0 = wp.tile([P, T], BF16, tag="GD0", name="GD0")
            nc.vector.tensor_tensor(GD0[:], pg0[:], Dup0, op=ALU.mult)
            nc.vector.tensor_tensor(GD1[:, P:T], pg1[:, P:T], Dup1r,
                                    op=ALU.mult)

            # ---- W = ln(emb rows)  (early, overlaps U/chain) -------------
            W = []
            for mt in range(2):
                st6 = wp.tile([P, 6], F32, tag=f"wst{mt}", name=f"wst{mt}")
                nc.vector.bn_stats(st6[:], emb_s[mt])
                mv = wp.tile([P, 2], F32, tag=f"wmv{mt}", name=f"wmv{mt}")
                nc.vector.bn_aggr(mv[:], st6[:])
                lv = wp.tile([P, 1], F32, tag=f"wlv{mt}", name=f"wlv{mt}")
                nc.scalar.activation(out=lv[:], in_=mv[:, 1:2], func=AF.Ln,
                                     bias=eps_col[:])
                rs = wp.tile([P, 1], F32, tag=f"wrs{mt}", name=f"wrs{mt}")
                nc.scalar.activation(out=rs[:], in_=lv[:], func=AF.Exp,
                                     bias=zero_col[:], scale=-0.5)
                w = wp.tile([P, D], BF16, tag=f"W{mt}", name=f"W{mt}")
                nc.vector.tensor_scalar(w[:], emb_s[mt], mv[:, 0:1], rs[:],
                                        op0=ALU.subtract, op1=ALU.mult)
                W.append(w)

            # ---- A^T directly (no transposes); pa for var only -----------
            ATp = [ps256.tile([P, T], F32, tag="ps", name=f"ATp{dt}")
                   for dt in range(2)]
            for dt in range(2):
                nc.tensor.matmul(ATp[dt][:], W[0][:, dt * P:(dt + 1) * P],
                                 GD0[:], start=True, stop=False)
                nc.tensor.matmul(ATp[dt][:], W[1][:, dt * P:(dt + 1) * P],
                                 GD1[:], start=False, stop=True)
            pa = [ps256.tile([P, D], F32, tag="ps", name=f"pa{mt}")
                  for mt in range(2)]
            nc.tensor.matmul(pa[0][:], GD0[:, 0:P], W[0][:],
                             start=True, stop=True)
            nc.tensor.matmul(pa[1][:], GD0[:, P:T], W[0][:],
                             start=True, stop=False)
            nc.tensor.matmul(pa[1][:], GD1[:, P:T], W[1][:],
                             start=False, stop=True)
            fillers(FILL_AT)

            AT = [wp.tile([P, T], BF16, tag=f"AT{dt}", name=f"AT{dt}")
                  for dt in range(2)]
            nc.scalar.copy(AT[0][:], ATp[0][:])
            nc.vector.tensor_copy(AT[1][:], ATp[1][:])

            # r_t = rsqrt(var(a*_t)+eps); mean(a*)==0, var=sum(a^2)/D via ACT
            r_col = []
            for mt in range(2):
                sq = wp.tile([P, D], F32, tag=f"asq{mt}", name=f"asq{mt}")
                ss = wp.tile([P, 1], F32, tag=f"ass{mt}", name=f"ass{mt}")
                nc.scalar.activation(out=sq[:], in_=pa[mt][:], func=AF.Square,
                                     bias=zero_col[:], accum_out=ss[:])
                lv = wp.tile([P, 1], F32, tag=f"alv{mt}", name=f"alv{mt}")
                nc.scalar.activation(out=lv[:], in_=ss[:], func=AF.Ln,
                                     bias=eps_col[:], scale=1.0 / D)
                rr = wp.tile([P, 1], F32, tag=f"ar{mt}", name=f"ar{mt}")
                nc.scalar.activation(out=rr[:], in_=lv[:], func=AF.Exp,
                                     bias=zero_col[:], scale=-0.5)
                r_col.append(rr)
            r2_col = []
            for mt in range(2):
                r2 = wp.tile([P, 1], F32, tag=f"r2{mt}", name=f"r2{mt}")
                nc.vector.tensor_tensor(r2[:], r_col[mt][:], r_col[mt][:],
                                        op=ALU.mult)
                r2_col.append(r2)

            # ---- y^T = relu(Dy A^T) o X^T --------------------------------
            yT = [wp.tile([P, T], BF16, tag=f"yT{m}", name=f"yT{m}")
                  for m in range(8)]
            for m in range(8):
                py = ps256.tile([P, T], F32, tag="ps", name=f"py{m}")
                for k in range(2):
                    nc.tensor.matmul(py[:], DyT[k][:, m * P:(m + 1) * P],
                                     AT[k][:], start=(k == 0), stop=(k == 1))
                if m % 2 == 0:
                    yb = wp.tile([P, T], BF16, tag=f"yb{m}", name=f"yb{m}")
                    nc.scalar.activation(out=yb[:], in_=py[:], func=AF.Relu,
                                         bias=zero_col[:])
                    nc.gpsimd.tensor_tensor(yT[m][:], yb[:], XT[m][:],
                                            op=ALU.mult)
                else:
                    nc.vector.scalar_tensor_tensor(
                        out=yT[m][:], in0=py[:], scalar=0.0, in1=XT[m][:],
                        op0=ALU.max, op1=ALU.mult)

            # ---- v = y E^T; apply r_t inside the output layernorm --------
            for mt in range(2):
                pv = ps256.tile([P, D], F32, tag="ps", name=f"pv{mt}")
                for m in range(8):
                    nc.tensor.matmul(pv[:], yT[m][:, mt * P:(mt + 1) * P],
                                     ET[m], start=(m == 0), stop=(m == 7))
                st6 = wp.tile([P, 6], F32, tag=f"ost{mt}", name=f"ost{mt}")
                nc.vector.bn_stats(st6[:], pv[:])
                mv = wp.tile([P, 2], F32, tag=f"omv{mt}", name=f"omv{mt}")
                nc.vector.bn_aggr(mv[:], st6[:])
                # out = (pv - mean) * r * rsqrt(r^2 var + eps)
                t1 = wp.tile([P, 1], F32, tag=f"ot1{mt}", name=f"ot1{mt}")
                nc.vector.tensor_tensor(t1[:], mv[:, 1:2], r2_col[mt][:],
                                        op=ALU.mult)
                lv = wp.tile([P, 1], F32, tag=f"olv{mt}", name=f"olv{mt}")
                nc.scalar.activation(out=lv[:], in_=t1[:], func=AF.Ln,
                                     bias=eps_col[:])
                rq = wp.tile([P, 1], F32, tag=f"orq{mt}", name=f"orq{mt}")
                nc.scalar.activation(out=rq[:], in_=lv[:], func=AF.Exp,
                                     bias=zero_col[:], scale=-0.5)
                s = wp.tile([P, 1], F32, tag=f"os{mt}", name=f"os{mt}")
                nc.vector.tensor_tensor(s[:], rq[:], r_col[mt][:],
                                        op=ALU.mult)
                ov = wp.tile([P, D], F32, tag=f"ov{mt}", name=f"ov{mt}")
                nc.vector.tensor_scalar(ov[:], pv[:], mv[:, 0:1], s[:],
                                        op0=ALU.subtract, op1=ALU.mult)
                eng_out = nc.sync if mt == 0 else nc.scalar
                eng_out.dma_start(d_out[mt * P:(mt + 1) * P, :], ov[:])

    nc.finalize()
    return nc


_NC_CACHE = {}


def _get_nc(use_f32r=True):
    if "nc" not in _NC_CACHE:
        _NC_CACHE["nc"] = _build_nc()
    return _NC_CACHE["nc"]


def _host_consts():
    ii = np.arange(T, dtype=np.float64)
    ln097 = np.log(np.float64(DECAY))
    DupT = np.where(
        ii[:, None] < ii[None, :],
        np.float64(DECAY) ** (ii[None, :] - 1 - ii[:, None]),
        0.0,
    ).astype(np.float32)
    jj = np.arange(P)
    triu = np.triu(np.ones((P, P), np.float32), k=1)
    ident = np.eye(P, dtype=np.float32)
    mtri = np.where(jj[:, None] <= jj[None, :], 0.0, -1e30).astype(np.float32)
    iotp = np.stack([(jj + mt * P) * ln097 for mt in range(2)],
                    axis=1).astype(np.float32)
    iotq = (-iotp).astype(np.float32)
    c097 = np.full((P, 2), DECAY, np.float32)
    c097[0, 0] = 0.0
    pack = np.concatenate(
        [DupT[0:P, 0:T], DupT[P:T, P:T], triu, ident, mtri, iotp, iotq,
         c097], axis=1)
    assert pack.shape == (P, DUPW), pack.shape
    return np.ascontiguousarray(pack)


def make_in_maps(embeddings, E, Dx, Dy):
    emb = np.asarray(embeddings, dtype=np.float32)
    E = np.asarray(E, dtype=np.float32)
    Dx = np.asarray(Dx, dtype=np.float32)
    Dy = np.asarray(Dy, dtype=np.float32)
    dup_pack = _host_consts()
    DxT = Dx.T  # [d, n]
    DyTp = np.ascontiguousarray(
        Dy.T.reshape(2, P, N).transpose(1, 0, 2).reshape(P, 2 * N)
    ).astype(NPBF16)
    ETp = np.ascontiguousarray(
        E.T.reshape(8, P, D).transpose(1, 0, 2).reshape(P, 8 * D)
    ).astype(NPBF16)
    shared = {"dyt": DyTp, "et": ETp, "dup": dup_pack}
    in_maps = []
    for b in range(B):
        m = dict(shared)
        embT = emb[b].T  # [d, t]
        for k in range(2):
            m[f"embdx{k}"] = np.ascontiguousarray(np.concatenate(
                [embT[k * P:(k + 1) * P, :], DxT[k * P:(k + 1) * P, :]],
                axis=1)).astype(NPBF16)
        m["emb"] = np.ascontiguousarray(
            emb[b].reshape(2, P, D).transpose(1, 0, 2).reshape(P, 2 * D)
        ).astype(NPBF16)
        in_maps.append(m)
    return in_maps


def kernel(embeddings, E, Dx, Dy, _use_f32r=True):
    in_maps = make_in_maps(embeddings, E, Dx, Dy)
    nc = _get_nc()
    res = run_bass_kernel_spmd(nc, in_maps, core_ids=list(range(B)))
    return np.stack([r["out"] for r in res.results], axis=0)
